# revision 30
# baseline (speedup 1.0000x reference)
"""Distributed MultiHeadAttention kernel for 8 TRN2 NeuronCores.

Problem: B=4, S=2048, D=1024, H=16, DH=64, fp32 reference, full
(non-causal) attention. ~137 GFLOP total.

Sharding (head-tensor-parallel x batch): core c owns batch b=c//2 and
head-half hh=c%2 (8 heads, full 2048-query x 2048-key attention). This
removes ALL duplicated work: each core projects q/k/v only for its 8
heads and contracts the output projection over its own 512 aot dims,
producing a partial Y[2048,1024] that the host sums pairwise
(Y(b) = Y(2b) + Y(2b+1) + bo). All 8 cores run ONE identical program;
only the DRAM inputs differ (batch xT and per-head-half weight slices).

Schedule notes (what made it fast, in order of impact):
- The attention inner loop is scalar(ACT)-bound: the per-key-chunk Exp
  [128,1024] takes ~1.1us while the 4 matmuls take ~0.86us. PV is
  software-pipelined ONE key chunk behind scores so the tensor queue
  never sits in the scores->exp->PV latency chain.
- The softmax reciprocal runs on the DVE (nc.vector.reciprocal), not as
  scalar Ln+Exp: the scalar engine is the attention bottleneck and the
  Ln table swap + 2 extra ACT ops per q2 cost more than DVE time.
- Pair j+1's Q/K projection matmuls are interleaved into pair j's
  attention stream (2 of 8 PSUM tiles after each q2 block) so the
  tensor engine uses the slack the scalar engine forces on it.
- aoT-normalize copies and Y staging copies run on GpSimd, which is
  otherwise idle after the x^T DMA; the DVE keeps the per-q2 tail
  (sum-row copies, reciprocal, normalize muls) short.
- fp8 was evaluated and rejected: absmax rel err tolerance (2e-2)
  needs per-element output noise sigma < 3e-3, and every fp8 placement
  (P/V, projections, or O-proj operands) measured 3e-2..9e-2 in a host
  simulation of the exact quantization chain.
- walrus in this environment rejects >1 semaphore wait per instruction;
  a post-pass hoists extra waits onto standalone same-engine
  InstEventSemaphore instructions.
"""
import numpy as np
import ml_dtypes
import concourse.bass as bass
import concourse.mybir as mybir
from concourse.tile import TileContext
from concourse.bass_utils import run_bass_kernel_spmd


def _ensure_trace_shim():
    """concourse's axon trace path imports antenv.axon_hooks, which this
    container's antenv lacks. Install a working ctypes-based NTFF hook (or a
    None hook) so BASS_TRACE=1 degrades gracefully instead of crashing."""
    try:
        import antenv.axon_hooks  # noqa: F401
        return
    except ImportError:
        pass
    import sys as _sys
    import types as _types
    hook = None
    try:
        if "/root/.axon_site" not in _sys.path:
            _sys.path.insert(0, "/root/.axon_site")
        from trn_agent_boot.trn_boot import _ntff_profile_via_ctypes
        hook = _ntff_profile_via_ctypes("/opt/axon/libaxon_pjrt.so")
    except Exception:
        hook = None
    mod = _types.ModuleType("antenv.axon_hooks")
    mod.get_axon_ntff_profile_hook = lambda: hook
    mod.set_axon_ntff_profile_hook = lambda h: None
    _sys.modules["antenv.axon_hooks"] = mod
    try:
        import concourse.bass_utils as _bu
        _bu.upload_artifacts = lambda tmpdir: f"local:{tmpdir}"
    except Exception:
        pass


_ensure_trace_shim()


F32 = mybir.dt.float32
F32R = mybir.dt.float32r
BF16 = mybir.dt.bfloat16
FP16 = mybir.dt.float16

B, S, D, H = 4, 2048, 1024, 16
DH = D // H
N_CORES = 8
NQ = S                     # 2048 queries per core (full sequence)
PAIRS = 4                  # head pairs per core (8 heads)
DINC = 8                   # 128-wide din chunks
KC = S // 128              # 16 key chunks
QT = NQ // 512             # 4 query tiles
SEG = 772                  # vaug cols per key chunk (4 pairs x 193)

_ws_counter = 0


def _split_multi_waits(nc):
    """walrus in this env rejects >1 sem wait per instruction; hoist extras
    onto same-engine standalone semaphore-wait instructions."""
    global _ws_counter
    f = nc.m.functions[0]
    for bb in f.blocks:
        insts = bb.instructions  # live list
        i = 0
        while i < len(insts):
            inst = insts[i]
            si = inst.sync_info
            waits = list(si.on_wait) if si is not None and si.on_wait else []
            if len(waits) > 1:
                eng = getattr(inst, "engine", None)
                assert eng is not None and eng in nc.engines, (
                    f"multi-wait on non-engine inst {inst.name} ({type(inst).__name__})"
                )
                for w in waits[:-1]:
                    _ws_counter += 1
                    ev = mybir.InstEventSemaphore(
                        name=f"I-wsplit-{_ws_counter}", ins=[], outs=[]
                    )
                    ev.engine = eng
                    ev.sync_info = mybir.SyncInfo(on_wait=[w], on_update=[])
                    nc.register_instruction(ev, overwrite=True)
                    insts.insert(i, ev)
                    i += 1
                inst.sync_info = mybir.SyncInfo(
                    on_wait=[waits[-1]], on_update=list(si.on_update or [])
                )
            i += 1


def build_bass():
    nc = bass.Bass()
    XT = nc.declare_dram_parameter("XT", [D, S], FP16, isOutput=False)
    WQP = nc.declare_dram_parameter("WQP", [PAIRS, 128, 1024], FP16, isOutput=False)
    WKP = nc.declare_dram_parameter("WKP", [PAIRS, 128, 1024], FP16, isOutput=False)
    WVP = nc.declare_dram_parameter("WVP", [128, 4096], FP16, isOutput=False)
    WOP = nc.declare_dram_parameter("WOP", [2, 128, 2048], FP16, isOutput=False)
    BQK = nc.declare_dram_parameter("BQK", [128, 8], F32, isOutput=False)
    BVB = nc.declare_dram_parameter("BVB", [128, 512], F32, isOutput=False)
    ONES2D = nc.declare_dram_parameter("ONES2D", [128, 128], F32, isOutput=False)
    # BMASK rows 0/32: partition-block masks so ONE K=33 matmul broadcasts
    # both reciprocal rows (rrec rows 0 and 32) to partitions 0:64 / 64:128.
    BMASK = nc.declare_dram_parameter("BMASK", [33, 128], F32, isOutput=False)
    Y = nc.declare_dram_parameter("Y", [NQ, D], FP16, isOutput=True)

    with TileContext(nc) as tc:
        with (
            tc.tile_pool(name="sb", bufs=1) as sb,
            tc.tile_pool(name="ps", bufs=1, space="PSUM") as ps,
        ):
            # ---- DMA priority: wv half 1 (first compute) on sync; x^T first
            # wave on gpsimd in parallel; everything else behind them.
            wv_t = sb.tile([128, 4096], FP16, tag="wv", name="wv_t")
            nc.sync.dma_start(out=wv_t[:, 0:2048], in_=WVP[:, 0:2048])

            xt = []
            for d in range(DINC):
                t = sb.tile([128, S], FP16, tag=f"xt{d}")
                nc.gpsimd.dma_start(out=t[:, 0:256],
                                    in_=XT[d * 128:(d + 1) * 128, 0:256])
                xt.append(t)
            nc.sync.dma_start(out=wv_t[:, 2048:4096], in_=WVP[:, 2048:4096])

            ones2d = sb.tile([128, 128], F32R, tag="ones2d")
            bmask = sb.tile([33, 128], F32R, tag="bmask")
            bqk = sb.tile([128, 8], F32, tag="bqk")
            bvb = sb.tile([128, 512], F32, tag="bvb")
            nc.sync.dma_start(out=bvb[:, :], in_=BVB[:, :])
            nc.sync.dma_start(out=bqk[:, :], in_=BQK[:, :])
            nc.sync.dma_start(out=ones2d[:, :], in_=ONES2D[:, :].bitcast(F32R))
            nc.sync.dma_start(out=bmask[:, :], in_=BMASK[:, :].bitcast(F32R))

            for d in range(DINC):
                nc.gpsimd.dma_start(out=xt[d][:, 256:1024],
                                    in_=XT[d * 128:(d + 1) * 128, 256:1024])
            for d in range(DINC):
                nc.gpsimd.dma_start(out=xt[d][:, 1024:2048],
                                    in_=XT[d * 128:(d + 1) * 128, 1024:2048])

            # Q/K/O weights early on the sync queue (it is idle after startup)
            wq_ts, wk_ts = [], []
            for j in range(PAIRS):
                wq_t = sb.tile([128, 1024], FP16, tag=f"wq{j}")
                wk_t = sb.tile([128, 1024], FP16, tag=f"wk{j}")
                nc.sync.dma_start(out=wq_t[:, :], in_=WQP[j, :, :])
                nc.sync.dma_start(out=wk_t[:, :], in_=WKP[j, :, :])
                wq_ts.append(wq_t)
                wk_ts.append(wk_t)
            wo_ts = []
            for nt in range(2):
                wo_t = sb.tile([128, 2048], FP16, tag=f"wo{nt}")
                nc.sync.dma_start(out=wo_t[:, :], in_=WOP[nt, :, :])
                wo_ts.append(wo_t)

            aot = [sb.tile([128, NQ], FP16, tag=f"ao{j}", name=f"ao{j}")
                   for j in range(PAIRS)]
            # pairs 0+1 partial of the output projection, staged in fp16;
            # computed one matmul per key chunk DURING pair 3's attention
            y01 = sb.tile([128, 32 * 512], FP16, tag="y01", name="y01")
            qt_ts = [sb.tile([128, NQ], FP16, tag=f"qt{j}", name=f"qt{j}") for j in range(PAIRS)]
            kt_ts = [sb.tile([128, S], FP16, tag=f"kt{j}", name=f"kt{j}") for j in range(PAIRS)]

            # ---- V-projection into augmented per-pair layout: per key-chunk
            # segment of 772 cols, 4 pair-sub-segments of 193:
            #   [V_h0 64 | ones | ones | junk62 | V_h1 64]
            # augA = seg[0:65]  (M=65; psA row 64 = h0 softmax sums)
            # augB = seg[65:193] (col 0 ones -> psB row 0 = h1 sums;
            #                     cols 64:128 = V_h1 -> psB rows 64:127)
            vaug = sb.tile([128, KC * SEG], BF16, tag="vaug", name="vaug")
            vsegs = vaug[:, :].rearrange("p (s c) -> p s c", c=SEG)
            for jj in range(PAIRS):
                nc.vector.memset(vsegs[:, :, jj * 193 + 64:jj * 193 + 65], 1.0)
                nc.vector.memset(vsegs[:, :, jj * 193 + 65:jj * 193 + 66], 1.0)
            for kc in range(KC):
                vps = ps.tile([128, 512], F32, tag="ps_proj", bufs=2)
                for d in range(DINC):
                    nc.tensor.matmul(
                        vps[:, :],
                        xt[d][:, kc * 128:(kc + 1) * 128],
                        wv_t[:, d * 512:(d + 1) * 512],
                        start=(d == 0), stop=(d == DINC - 1),
                    )
                # merged strided adds: one op per head-position covers all 4
                # pair sub-segments (3D APs), 2 DVE ops per key chunk not 8
                s0 = kc * SEG
                vseg = vaug[:, s0:s0 + SEG].rearrange("p (j c) -> p j c", c=193)
                vpsq = vps[:, :].rearrange("p (j c) -> p j c", c=128)
                bvbq = bvb[:, :].rearrange("p (j c) -> p j c", c=128)
                with nc.allow_low_precision(reason="bf16 V"):
                    nc.vector.tensor_add(
                        vseg[:, :, 0:64], vpsq[:, :, 0:64], bvbq[:, :, 0:64])
                    nc.vector.tensor_add(
                        vseg[:, :, 129:193], vpsq[:, :, 64:128],
                        bvbq[:, :, 64:128])

            def qk_proj_tile(j, which, tt, eng=None):
                """One PSUM tile (1/8) of pair j's Q or K projection. The
                bias-add runs on GpSimd when interleaved into the attention
                stream: the DVE's 2-4.6us reciprocal blocks would otherwise
                delay the add, park the ps_proj buffer, and stall the whole
                (scheduler-interleaved) tensor queue."""
                w_t = wq_ts[j] if which == 0 else wk_ts[j]
                o_t = qt_ts[j] if which == 0 else kt_ts[j]
                pps = ps.tile([128, 512], F32, tag="ps_proj", bufs=2)
                for d in range(DINC):
                    nc.tensor.matmul(
                        pps[:, :],
                        w_t[:, d * 128:(d + 1) * 128],
                        xt[d][:, tt * 512:(tt + 1) * 512],
                        start=(d == 0), stop=(d == DINC - 1),
                    )
                with nc.allow_low_precision(reason="f32r rounding"):
                    (eng or nc.vector).tensor_scalar_add(
                        o_t[:, tt * 512:(tt + 1) * 512], pps[:, :],
                        bqk[:, 2 * j + which:2 * j + which + 1],
                    )

            # pair 0's full Q/K projection up front
            for which in range(2):
                for tt in range(4):
                    qk_proj_tile(0, which, tt)

            # ---- main loop over head pairs; pair j+1's projection is
            # interleaved (2 of its 8 PSUM tiles after each q2 block), and
            # each q2's softmax-tail broadcast+muls are DEFERRED into the
            # middle of the next q2's kc loop: by then the DVE reciprocal is
            # long finished, so the in-order tensor queue never waits on it.
            def make_tail(j, qsl, rrec, aocp):
                def emit():
                    # single-bank broadcast: ONE K=33 matmul with the BMASK
                    # stationary places 1/sums_h0 on partitions 0:64 and
                    # 1/sums_h1 on 64:128 (junk rrec rows hit zero mask
                    # rows).  psbc comes from the ps_proj pool: its rotation
                    # gives the muls ~10us before the bank is reused (the
                    # ps_s rotation would stall the next q2's scores).
                    psbc = ps.tile([128, 512], F32, tag="ps_proj", bufs=2,
                                   name="psbc")
                    nc.tensor.matmul(psbc[:, :], bmask[:, :], rrec[:, :],
                                     start=True, stop=True)
                    with nc.allow_low_precision(reason="fp16 out"):
                        nc.vector.tensor_mul(
                            aot[j][0:64, qsl], aocp[0:64, 0:512],
                            psbc[0:64, :],
                        )
                        nc.vector.tensor_mul(
                            aot[j][64:128, qsl], aocp[64:128, 512:1024],
                            psbc[64:128, :],
                        )
                return emit

            pending_tail = None
            for j in range(PAIRS):
                qt_t, kt_t = qt_ts[j], kt_ts[j]
                for q2 in range(QT):
                    psA = ps.tile([65, 512], F32, tag="ps_pv", bufs=2)
                    psB = ps.tile([128, 512], F32, tag="ps_pv", bufs=2)
                    qsl = slice(q2 * 512, (q2 + 1) * 512)
                    # software-pipelined: scores(kc) ... PV(kc-1); one matmul
                    # of the next pair's projection rides in each kc period
                    # (tile a over kc 1..8, tile b over kc 9..16).
                    pts = [None, None]
                    proj_plan = []
                    if j + 1 < PAIRS:
                        for half in range(2):
                            w_t = wq_ts[j + 1] if q2 < 2 else wk_ts[j + 1]
                            o_t = qt_ts[j + 1] if q2 < 2 else kt_ts[j + 1]
                            tt = 2 * (q2 % 2) + half
                            proj_plan.append(
                                (w_t, o_t, tt, 2 * (j + 1) + (0 if q2 < 2 else 1)))
                    proj_ps = [None]

                    def proj_step(step):
                        half, d = step // DINC, step % DINC
                        w_t, o_t, tt, bcol = proj_plan[half]
                        if d == 0:
                            proj_ps[0] = ps.tile([128, 512], F32,
                                                 tag="ps_proj", bufs=2,
                                                 name="pps")
                        nc.tensor.matmul(
                            proj_ps[0][:, :],
                            w_t[:, d * 128:(d + 1) * 128],
                            xt[d][:, tt * 512:(tt + 1) * 512],
                            start=(d == 0), stop=(d == DINC - 1),
                        )
                        if d == DINC - 1:
                            with nc.allow_low_precision(reason="f32r rounding"):
                                nc.vector.tensor_scalar_add(
                                    o_t[:, tt * 512:(tt + 1) * 512],
                                    proj_ps[0][:, :],
                                    bqk[:, bcol:bcol + 1],
                                )

                    def oproj1_step(step):
                        # pair-3 interleave: pairs 0+1 of the output
                        # projection, one matmul per key chunk -> y01
                        t = q2 * 8 + step // 2
                        jm = step % 2
                        nt, tc_ = t // 16, t % 16
                        if jm == 0:
                            proj_ps[0] = ps.tile([128, 512], F32,
                                                 tag="ps_proj", bufs=2,
                                                 name="yps1")
                        nc.tensor.matmul(
                            proj_ps[0][:, :],
                            aot[jm][:, tc_ * 128:(tc_ + 1) * 128],
                            wo_ts[nt][:, jm * 512:(jm + 1) * 512],
                            start=(jm == 0), stop=(jm == 1),
                        )
                        if jm == 1:
                            with nc.allow_low_precision(reason="fp16 partial"):
                                nc.vector.tensor_copy(
                                    y01[:, t * 512:(t + 1) * 512],
                                    proj_ps[0][:, :])

                    for kc in range(KC + 1):
                        if kc == 6 and pending_tail is not None:
                            pending_tail()
                            pending_tail = None
                        if kc < KC:
                            pss = ps.tile([128, 1024], F32, tag="ps_s", bufs=2)
                            ksl = slice(kc * 128, (kc + 1) * 128)
                            nc.tensor.matmul(
                                pss[:, 0:512], kt_t[0:64, ksl], qt_t[0:64, qsl],
                                start=True, stop=True,
                            )
                            nc.tensor.matmul(
                                pss[:, 512:1024], kt_t[64:128, ksl],
                                qt_t[64:128, qsl],
                                start=True, stop=True,
                            )
                            pt = sb.tile([128, 1024], BF16, tag="pt", bufs=6)
                            nc.scalar.activation(
                                pt[:, :], pss[:, :],
                                mybir.ActivationFunctionType.Exp,
                            )
                            pts[kc % 2] = pt
                        if kc >= 1:
                            pv = kc - 1
                            pt_p = pts[pv % 2]
                            s0 = pv * SEG + j * 193
                            nc.tensor.matmul(
                                psA[:, :], vaug[:, s0:s0 + 65], pt_p[:, 0:512],
                                start=(pv == 0), stop=(pv == KC - 1),
                            )
                            nc.tensor.matmul(
                                psB[:, :], vaug[:, s0 + 65:s0 + 193],
                                pt_p[:, 512:1024],
                                start=(pv == 0), stop=(pv == KC - 1),
                            )
                            if proj_plan:
                                proj_step(kc - 1)
                            elif j == PAIRS - 1:
                                oproj1_step(kc - 1)

                    # DVE half of the softmax tail, emitted now: free psA/psB
                    # via the staging copies, then 1/sums on the DVE (the
                    # [33,512] reciprocal costs ~3.3us and runs during the
                    # next q2's early key chunks).  sums sit in psA row 64
                    # (h0, copied to row 0) / psB row 0 (h1, copied to row
                    # 32); srow rows 1..31 are memset to 1.0 once per buffer
                    # so the reciprocal reads no junk.
                    srow = sb.tile([33, 512], F32, tag="srow", bufs=2)
                    rrec = sb.tile([33, 512], F32R, tag="rrec", bufs=2)
                    aocp = sb.tile([128, 1024], F32, tag="aocp", bufs=2)
                    nc.vector.tensor_copy(aocp[0:64, 0:512], psA[0:64, :])
                    nc.vector.tensor_copy(aocp[64:128, 512:1024], psB[64:128, :])
                    if j == 0 and q2 < 2:
                        nc.vector.memset(srow[:, :], 1.0)
                    nc.vector.tensor_copy(srow[0:1, :], psA[64:65, :])
                    nc.vector.tensor_copy(srow[32:33, :], psB[0:1, :])
                    with nc.allow_low_precision(reason="softmax recip"):
                        nc.vector.reciprocal(out=rrec[:, :],
                                             in_=srow[:, :])
                    pending_tail = make_tail(j, qsl, rrec, aocp)

            # ---- output projection pass 2: pairs 2+3 accumulate in PSUM,
            # then y01 (pairs 0+1, staged during pair 3's attention) is added
            # on the DVE and Y goes out in fp16 (halves the DMA drain; the
            # host sums the two core partials in f32).  The last q2's
            # deferred softmax tail is emitted a few tiles in; only token
            # chunks 12..15 depend on it.
            for nt in range(2):
                wo_t = wo_ts[nt]
                for tc_ in range(16):
                    if nt == 0 and tc_ == 4 and pending_tail is not None:
                        pending_tail()
                        pending_tail = None
                    t = nt * 16 + tc_
                    yps = ps.tile([128, 512], F32, tag="ps_proj", bufs=2)
                    for j in (2, 3):
                        nc.tensor.matmul(
                            yps[:, :],
                            aot[j][:, tc_ * 128:(tc_ + 1) * 128],
                            wo_t[:, j * 512:(j + 1) * 512],
                            start=(j == 2), stop=(j == 3),
                        )
                    y_sb = sb.tile([128, 512], FP16, tag="y", bufs=3)
                    with nc.allow_low_precision(reason="fp16 partial Y"):
                        nc.vector.tensor_add(
                            y_sb[:, :], yps[:, :],
                            y01[:, t * 512:(t + 1) * 512])
                    eng = nc.sync if tc_ % 2 == 0 else nc.gpsimd
                    eng.dma_start(
                        out=Y[tc_ * 128:(tc_ + 1) * 128, nt * 512:(nt + 1) * 512],
                        in_=y_sb[:, :],
                    )

    _split_multi_waits(nc)
    return nc


_nc_cache = {}
_last_results = None


def _get_nc():
    if "nc" not in _nc_cache:
        _nc_cache["nc"] = build_bass()
    return _nc_cache["nc"]


def _prep_weights(hh, wq, bq, wk, bk, wv, bv, wo):
    """Per-head-half (hh in {0,1}) weight pack. Global pairs hh*4..hh*4+3."""
    wqT = np.ascontiguousarray(wq.T) * np.float32(1.0 / np.sqrt(DH))
    wkT = np.ascontiguousarray(wk.T)
    wvT = np.ascontiguousarray(wv.T)
    woT = np.ascontiguousarray(wo.T)
    jsl = slice(hh * PAIRS, (hh + 1) * PAIRS)
    csl = slice(hh * 512, (hh + 1) * 512)
    # WQP[j, p, (d m)] = wqT[d*128+p, (hh*4+j)*128+m]
    A = wqT.reshape(DINC, 128, 2 * PAIRS, 128)
    WQP = np.ascontiguousarray(
        A.transpose(2, 1, 0, 3)[jsl].reshape(PAIRS, 128, 1024)).astype(np.float16)
    A = wkT.reshape(DINC, 128, 2 * PAIRS, 128)
    WKP = np.ascontiguousarray(
        A.transpose(2, 1, 0, 3)[jsl].reshape(PAIRS, 128, 1024)).astype(np.float16)
    # WVP[p, (d n)] = wvT[d*128+p, hh*512+n]
    A = wvT[:, csl].reshape(DINC, 128, 512)
    WVP = np.ascontiguousarray(
        A.transpose(1, 0, 2).reshape(128, 4096)).astype(np.float16)
    # WOP[nt, p, (j n)] = woT[hh*512 + j*128+p, nt*512+n]
    A = woT[csl].reshape(PAIRS, 128, 2, 512)
    WOP = np.ascontiguousarray(
        A.transpose(2, 1, 0, 3).reshape(2, 128, 2048)).astype(np.float16)
    bqs = (bq * np.float32(1.0 / np.sqrt(DH))).reshape(2 * PAIRS, 128)[jsl]
    bkr = bk.reshape(2 * PAIRS, 128)[jsl]
    BQK = np.empty((128, 8), np.float32)
    for jx in range(PAIRS):
        BQK[:, 2 * jx] = bqs[jx]
        BQK[:, 2 * jx + 1] = bkr[jx]
    BVB = np.ascontiguousarray(np.tile(bv[csl].reshape(1, 512), (128, 1)))
    return {"WQP": WQP, "WKP": WKP, "WVP": WVP, "WOP": WOP,
            "BQK": BQK, "BVB": BVB}


def kernel(x_input, wq, bq, wk, bk, wv, bv, wo, bo):
    x_input = np.asarray(x_input, dtype=np.float32)
    wq, bq = np.asarray(wq, np.float32), np.asarray(bq, np.float32)
    wk, bk = np.asarray(wk, np.float32), np.asarray(bk, np.float32)
    wv, bv = np.asarray(wv, np.float32), np.asarray(bv, np.float32)
    wo, bo = np.asarray(wo, np.float32), np.asarray(bo, np.float32)

    packs = [_prep_weights(hh, wq, bq, wk, bk, wv, bv, wo) for hh in range(2)]
    ONES2D = np.ones((128, 128), np.float32)
    BMASK = np.zeros((33, 128), np.float32)
    BMASK[0, 0:64] = 1.0
    BMASK[32, 64:128] = 1.0
    xTs = [np.ascontiguousarray(x_input[b].T).astype(np.float16) for b in range(B)]

    nc = _get_nc()
    in_maps = []
    for c in range(N_CORES):
        b, hh = c // 2, c % 2
        m = dict(packs[hh])
        m["XT"] = xTs[b]
        m["ONES2D"] = ONES2D
        m["BMASK"] = BMASK
        in_maps.append(m)

    res = run_bass_kernel_spmd(nc, in_maps, list(range(N_CORES)))
    global _last_results
    _last_results = res

    out = np.empty((B, S, D), np.float32)
    for b in range(B):
        out[b] = res.results[2 * b]["Y"].astype(np.float32)
        out[b] += res.results[2 * b + 1]["Y"].astype(np.float32)
    out += bo.reshape(1, 1, D)
    return out


# revision 34
# speedup vs baseline: 1.0219x; 1.0219x over previous
"""Distributed MultiHeadAttention kernel for 8 TRN2 NeuronCores.

Problem: B=4, S=2048, D=1024, H=16, DH=64, fp32 reference, full
(non-causal) attention. ~137 GFLOP total.

Sharding (head-tensor-parallel x batch): core c owns batch b=c//2 and
head-half hh=c%2 (8 heads, full 2048-query x 2048-key attention). This
removes ALL duplicated work: each core projects q/k/v only for its 8
heads and contracts the output projection over its own 512 aot dims,
producing a partial Y[2048,1024] that the host sums pairwise
(Y(b) = Y(2b) + Y(2b+1) + bo). All 8 cores run ONE identical program;
only the DRAM inputs differ (batch xT and per-head-half weight slices).

Schedule notes (what made it fast, in order of impact):
- The attention inner loop is scalar(ACT)-bound: the per-key-chunk Exp
  [128,1024] takes ~1.1us while the 4 matmuls take ~0.86us. PV is
  software-pipelined ONE key chunk behind scores so the tensor queue
  never sits in the scores->exp->PV latency chain.
- The softmax reciprocal runs on the DVE (nc.vector.reciprocal), not as
  scalar Ln+Exp: the scalar engine is the attention bottleneck and the
  Ln table swap + 2 extra ACT ops per q2 cost more than DVE time.
- Pair j+1's Q/K projection matmuls are interleaved into pair j's
  attention stream (2 of 8 PSUM tiles after each q2 block) so the
  tensor engine uses the slack the scalar engine forces on it.
- aoT-normalize copies and Y staging copies run on GpSimd, which is
  otherwise idle after the x^T DMA; the DVE keeps the per-q2 tail
  (sum-row copies, reciprocal, normalize muls) short.
- fp8 was evaluated and rejected: absmax rel err tolerance (2e-2)
  needs per-element output noise sigma < 3e-3, and every fp8 placement
  (P/V, projections, or O-proj operands) measured 3e-2..9e-2 in a host
  simulation of the exact quantization chain.
- walrus in this environment rejects >1 semaphore wait per instruction;
  a post-pass hoists extra waits onto standalone same-engine
  InstEventSemaphore instructions.
"""
import numpy as np
import ml_dtypes
import concourse.bass as bass
import concourse.mybir as mybir
from concourse.tile import TileContext
from concourse.bass_utils import run_bass_kernel_spmd


def _ensure_trace_shim():
    """concourse's axon trace path imports antenv.axon_hooks, which this
    container's antenv lacks. Install a working ctypes-based NTFF hook (or a
    None hook) so BASS_TRACE=1 degrades gracefully instead of crashing."""
    try:
        import antenv.axon_hooks  # noqa: F401
        return
    except ImportError:
        pass
    import sys as _sys
    import types as _types
    hook = None
    try:
        if "/root/.axon_site" not in _sys.path:
            _sys.path.insert(0, "/root/.axon_site")
        from trn_agent_boot.trn_boot import _ntff_profile_via_ctypes
        hook = _ntff_profile_via_ctypes("/opt/axon/libaxon_pjrt.so")
    except Exception:
        hook = None
    mod = _types.ModuleType("antenv.axon_hooks")
    mod.get_axon_ntff_profile_hook = lambda: hook
    mod.set_axon_ntff_profile_hook = lambda h: None
    _sys.modules["antenv.axon_hooks"] = mod
    try:
        import concourse.bass_utils as _bu
        _bu.upload_artifacts = lambda tmpdir: f"local:{tmpdir}"
    except Exception:
        pass


_ensure_trace_shim()


F32 = mybir.dt.float32
F32R = mybir.dt.float32r
BF16 = mybir.dt.bfloat16
FP16 = mybir.dt.float16

B, S, D, H = 4, 2048, 1024, 16
DH = D // H
N_CORES = 8
NQ = S                     # 2048 queries per core (full sequence)
PAIRS = 4                  # head pairs per core (8 heads)
DINC = 8                   # 128-wide din chunks
KC = S // 128              # 16 key chunks
QT = NQ // 512             # 4 query tiles
SEG = 772                  # vaug cols per key chunk (4 pairs x 193)

_ws_counter = 0


def _split_multi_waits(nc):
    """walrus in this env rejects >1 sem wait per instruction; hoist extras
    onto same-engine standalone semaphore-wait instructions."""
    global _ws_counter
    f = nc.m.functions[0]
    for bb in f.blocks:
        insts = bb.instructions  # live list
        i = 0
        while i < len(insts):
            inst = insts[i]
            si = inst.sync_info
            waits = list(si.on_wait) if si is not None and si.on_wait else []
            if len(waits) > 1:
                eng = getattr(inst, "engine", None)
                assert eng is not None and eng in nc.engines, (
                    f"multi-wait on non-engine inst {inst.name} ({type(inst).__name__})"
                )
                for w in waits[:-1]:
                    _ws_counter += 1
                    ev = mybir.InstEventSemaphore(
                        name=f"I-wsplit-{_ws_counter}", ins=[], outs=[]
                    )
                    ev.engine = eng
                    ev.sync_info = mybir.SyncInfo(on_wait=[w], on_update=[])
                    nc.register_instruction(ev, overwrite=True)
                    insts.insert(i, ev)
                    i += 1
                inst.sync_info = mybir.SyncInfo(
                    on_wait=[waits[-1]], on_update=list(si.on_update or [])
                )
            i += 1


def build_bass():
    nc = bass.Bass()
    XT = nc.declare_dram_parameter("XT", [D, S], FP16, isOutput=False)
    WQP = nc.declare_dram_parameter("WQP", [PAIRS, 128, 1024], FP16, isOutput=False)
    WKP = nc.declare_dram_parameter("WKP", [PAIRS, 128, 1024], FP16, isOutput=False)
    WVP = nc.declare_dram_parameter("WVP", [128, 4096], FP16, isOutput=False)
    WOP = nc.declare_dram_parameter("WOP", [2, 128, 2048], FP16, isOutput=False)
    BQK = nc.declare_dram_parameter("BQK", [128, 8], F32, isOutput=False)
    BVB = nc.declare_dram_parameter("BVB", [128, 512], F32, isOutput=False)
    ONES2D = nc.declare_dram_parameter("ONES2D", [128, 128], F32, isOutput=False)
    # BMASK rows 0/32: partition-block masks so ONE K=33 matmul broadcasts
    # both reciprocal rows (rrec rows 0 and 32) to partitions 0:64 / 64:128.
    BMASK = nc.declare_dram_parameter("BMASK", [33, 128], F32, isOutput=False)
    Y = nc.declare_dram_parameter("Y", [NQ, D], FP16, isOutput=True)

    with TileContext(nc) as tc:
        with (
            tc.tile_pool(name="sb", bufs=1) as sb,
            tc.tile_pool(name="ps", bufs=1, space="PSUM") as ps,
        ):
            # ---- DMA priority: wv half 1 (first compute) on sync; x^T first
            # wave on gpsimd in parallel; everything else behind them.
            wv_t = sb.tile([128, 4096], FP16, tag="wv", name="wv_t")
            nc.sync.dma_start(out=wv_t[:, 0:2048], in_=WVP[:, 0:2048])

            xt = []
            for d in range(DINC):
                t = sb.tile([128, S], FP16, tag=f"xt{d}")
                nc.gpsimd.dma_start(out=t[:, 0:256],
                                    in_=XT[d * 128:(d + 1) * 128, 0:256])
                xt.append(t)
            nc.sync.dma_start(out=wv_t[:, 2048:4096], in_=WVP[:, 2048:4096])

            ones2d = sb.tile([128, 128], F32R, tag="ones2d")
            bmask = sb.tile([33, 128], F32R, tag="bmask")
            bqk = sb.tile([128, 8], F32, tag="bqk")
            bvb = sb.tile([128, 512], F32, tag="bvb")
            nc.sync.dma_start(out=bvb[:, :], in_=BVB[:, :])
            nc.sync.dma_start(out=bqk[:, :], in_=BQK[:, :])
            nc.sync.dma_start(out=ones2d[:, :], in_=ONES2D[:, :].bitcast(F32R))
            nc.sync.dma_start(out=bmask[:, :], in_=BMASK[:, :].bitcast(F32R))

            for d in range(DINC):
                nc.gpsimd.dma_start(out=xt[d][:, 256:1024],
                                    in_=XT[d * 128:(d + 1) * 128, 256:1024])
            for d in range(DINC):
                nc.gpsimd.dma_start(out=xt[d][:, 1024:2048],
                                    in_=XT[d * 128:(d + 1) * 128, 1024:2048])

            # Q/K/O weights early on the sync queue (it is idle after startup)
            wq_ts, wk_ts = [], []
            for j in range(PAIRS):
                wq_t = sb.tile([128, 1024], FP16, tag=f"wq{j}")
                wk_t = sb.tile([128, 1024], FP16, tag=f"wk{j}")
                nc.sync.dma_start(out=wq_t[:, :], in_=WQP[j, :, :])
                nc.sync.dma_start(out=wk_t[:, :], in_=WKP[j, :, :])
                wq_ts.append(wq_t)
                wk_ts.append(wk_t)
            wo_ts = []
            for nt in range(2):
                wo_t = sb.tile([128, 2048], FP16, tag=f"wo{nt}")
                nc.sync.dma_start(out=wo_t[:, :], in_=WOP[nt, :, :])
                wo_ts.append(wo_t)

            aot = [sb.tile([128, NQ], FP16, tag=f"ao{j}", name=f"ao{j}")
                   for j in range(PAIRS)]
            qt_ts = [sb.tile([128, NQ], FP16, tag=f"qt{j}", name=f"qt{j}") for j in range(PAIRS)]
            kt_ts = [sb.tile([128, S], FP16, tag=f"kt{j}", name=f"kt{j}") for j in range(PAIRS)]

            # ---- V-projection into augmented per-pair layout: per key-chunk
            # segment of 772 cols, 4 pair-sub-segments of 193:
            #   [V_h0 64 | ones | ones | junk62 | V_h1 64]
            # augA = seg[0:65]  (M=65; psA row 64 = h0 softmax sums)
            # augB = seg[65:193] (col 0 ones -> psB row 0 = h1 sums;
            #                     cols 64:128 = V_h1 -> psB rows 64:127)
            vaug = sb.tile([128, KC * SEG], BF16, tag="vaug", name="vaug")
            vsegs = vaug[:, :].rearrange("p (s c) -> p s c", c=SEG)
            for jj in range(PAIRS):
                nc.vector.memset(vsegs[:, :, jj * 193 + 64:jj * 193 + 65], 1.0)
                nc.vector.memset(vsegs[:, :, jj * 193 + 65:jj * 193 + 66], 1.0)
            for kc in range(KC):
                vps = ps.tile([128, 512], F32, tag="ps_proj", bufs=2)
                for d in range(DINC):
                    nc.tensor.matmul(
                        vps[:, :],
                        xt[d][:, kc * 128:(kc + 1) * 128],
                        wv_t[:, d * 512:(d + 1) * 512],
                        start=(d == 0), stop=(d == DINC - 1),
                    )
                # merged strided adds: one op per head-position covers all 4
                # pair sub-segments (3D APs), 2 DVE ops per key chunk not 8
                s0 = kc * SEG
                vseg = vaug[:, s0:s0 + SEG].rearrange("p (j c) -> p j c", c=193)
                vpsq = vps[:, :].rearrange("p (j c) -> p j c", c=128)
                bvbq = bvb[:, :].rearrange("p (j c) -> p j c", c=128)
                with nc.allow_low_precision(reason="bf16 V"):
                    nc.vector.tensor_add(
                        vseg[:, :, 0:64], vpsq[:, :, 0:64], bvbq[:, :, 0:64])
                    nc.vector.tensor_add(
                        vseg[:, :, 129:193], vpsq[:, :, 64:128],
                        bvbq[:, :, 64:128])

            def qk_proj_tile(j, which, tt, eng=None):
                """One PSUM tile (1/8) of pair j's Q or K projection. The
                bias-add runs on GpSimd when interleaved into the attention
                stream: the DVE's 2-4.6us reciprocal blocks would otherwise
                delay the add, park the ps_proj buffer, and stall the whole
                (scheduler-interleaved) tensor queue."""
                w_t = wq_ts[j] if which == 0 else wk_ts[j]
                o_t = qt_ts[j] if which == 0 else kt_ts[j]
                pps = ps.tile([128, 512], F32, tag="ps_proj", bufs=2)
                for d in range(DINC):
                    nc.tensor.matmul(
                        pps[:, :],
                        w_t[:, d * 128:(d + 1) * 128],
                        xt[d][:, tt * 512:(tt + 1) * 512],
                        start=(d == 0), stop=(d == DINC - 1),
                    )
                with nc.allow_low_precision(reason="f32r rounding"):
                    (eng or nc.vector).tensor_scalar_add(
                        o_t[:, tt * 512:(tt + 1) * 512], pps[:, :],
                        bqk[:, 2 * j + which:2 * j + which + 1],
                    )

            # pair 0's full Q/K projection up front
            for which in range(2):
                for tt in range(4):
                    qk_proj_tile(0, which, tt)

            # ---- main loop over head pairs; pair j+1's projection is
            # interleaved (2 of its 8 PSUM tiles after each q2 block), and
            # each q2's softmax-tail broadcast+muls are DEFERRED into the
            # middle of the next q2's kc loop: by then the DVE reciprocal is
            # long finished, so the in-order tensor queue never waits on it.
            def make_tail(j, qsl, rrec, aocp):
                def emit():
                    # single-bank broadcast: ONE K=33 matmul with the BMASK
                    # stationary places 1/sums_h0 on partitions 0:64 and
                    # 1/sums_h1 on 64:128 (junk rrec rows hit zero mask
                    # rows).  psbc comes from the ps_proj pool: its rotation
                    # gives the muls ~10us before the bank is reused (the
                    # ps_s rotation would stall the next q2's scores).
                    psbc = ps.tile([128, 512], F32, tag="ps_proj", bufs=2,
                                   name="psbc")
                    nc.tensor.matmul(psbc[:, :], bmask[:, :], rrec[:, :],
                                     start=True, stop=True)
                    with nc.allow_low_precision(reason="fp16 out"):
                        nc.vector.tensor_mul(
                            aot[j][0:64, qsl], aocp[0:64, 0:512],
                            psbc[0:64, :],
                        )
                        nc.vector.tensor_mul(
                            aot[j][64:128, qsl], aocp[64:128, 512:1024],
                            psbc[64:128, :],
                        )
                return emit

            pending_tail = None
            for j in range(PAIRS):
                qt_t, kt_t = qt_ts[j], kt_ts[j]
                for q2 in range(QT):
                    psA = ps.tile([65, 512], F32, tag="ps_pv", bufs=2)
                    psB = ps.tile([128, 512], F32, tag="ps_pv", bufs=2)
                    qsl = slice(q2 * 512, (q2 + 1) * 512)
                    # software-pipelined: scores(kc) ... PV(kc-1); one matmul
                    # of the next pair's projection rides in each kc period
                    # (tile a over kc 1..8, tile b over kc 9..16).
                    pts = [None, None]
                    proj_plan = []
                    if j + 1 < PAIRS:
                        for half in range(2):
                            w_t = wq_ts[j + 1] if q2 < 2 else wk_ts[j + 1]
                            o_t = qt_ts[j + 1] if q2 < 2 else kt_ts[j + 1]
                            tt = 2 * (q2 % 2) + half
                            proj_plan.append(
                                (w_t, o_t, tt, 2 * (j + 1) + (0 if q2 < 2 else 1)))
                    proj_ps = [None]

                    def proj_step(step):
                        half, d = step // DINC, step % DINC
                        w_t, o_t, tt, bcol = proj_plan[half]
                        if d == 0:
                            proj_ps[0] = ps.tile([128, 512], F32,
                                                 tag="ps_proj", bufs=2,
                                                 name="pps")
                        nc.tensor.matmul(
                            proj_ps[0][:, :],
                            w_t[:, d * 128:(d + 1) * 128],
                            xt[d][:, tt * 512:(tt + 1) * 512],
                            start=(d == 0), stop=(d == DINC - 1),
                        )
                        if d == DINC - 1:
                            with nc.allow_low_precision(reason="f32r rounding"):
                                nc.vector.tensor_scalar_add(
                                    o_t[:, tt * 512:(tt + 1) * 512],
                                    proj_ps[0][:, :],
                                    bqk[:, bcol:bcol + 1],
                                )

                    for kc in range(KC + 1):
                        if kc == 6 and pending_tail is not None:
                            pending_tail()
                            pending_tail = None
                        if kc < KC:
                            pss = ps.tile([128, 1024], F32, tag="ps_s", bufs=2)
                            ksl = slice(kc * 128, (kc + 1) * 128)
                            nc.tensor.matmul(
                                pss[:, 0:512], kt_t[0:64, ksl], qt_t[0:64, qsl],
                                start=True, stop=True,
                            )
                            nc.tensor.matmul(
                                pss[:, 512:1024], kt_t[64:128, ksl],
                                qt_t[64:128, qsl],
                                start=True, stop=True,
                            )
                            pt = sb.tile([128, 1024], BF16, tag="pt", bufs=6)
                            nc.scalar.activation(
                                pt[:, :], pss[:, :],
                                mybir.ActivationFunctionType.Exp,
                            )
                            pts[kc % 2] = pt
                        if kc >= 1:
                            pv = kc - 1
                            pt_p = pts[pv % 2]
                            s0 = pv * SEG + j * 193
                            nc.tensor.matmul(
                                psA[:, :], vaug[:, s0:s0 + 65], pt_p[:, 0:512],
                                start=(pv == 0), stop=(pv == KC - 1),
                            )
                            nc.tensor.matmul(
                                psB[:, :], vaug[:, s0 + 65:s0 + 193],
                                pt_p[:, 512:1024],
                                start=(pv == 0), stop=(pv == KC - 1),
                            )
                            if proj_plan:
                                proj_step(kc - 1)

                    # DVE half of the softmax tail, emitted now: free psA/psB
                    # via the staging copies, then 1/sums on the DVE (the
                    # [33,512] reciprocal costs ~3.3us and runs during the
                    # next q2's early key chunks).  sums sit in psA row 64
                    # (h0, copied to row 0) / psB row 0 (h1, copied to row
                    # 32); srow rows 1..31 are memset to 1.0 once per buffer
                    # so the reciprocal reads no junk.
                    srow = sb.tile([33, 512], F32, tag="srow", bufs=2)
                    rrec = sb.tile([33, 512], F32R, tag="rrec", bufs=2)
                    aocp = sb.tile([128, 1024], F32, tag="aocp", bufs=2)
                    nc.vector.tensor_copy(aocp[0:64, 0:512], psA[0:64, :])
                    nc.vector.tensor_copy(aocp[64:128, 512:1024], psB[64:128, :])
                    if j == 0 and q2 < 2:
                        nc.vector.memset(srow[:, :], 1.0)
                    nc.vector.tensor_copy(srow[0:1, :], psA[64:65, :])
                    nc.vector.tensor_copy(srow[32:33, :], psB[0:1, :])
                    with nc.allow_low_precision(reason="softmax recip"):
                        nc.vector.reciprocal(out=rrec[:, :],
                                             in_=srow[:, :])
                    pending_tail = make_tail(j, qsl, rrec, aocp)

            # ---- output projection pass 2: pairs 2+3 accumulate in PSUM,
            # then y01 (pairs 0+1, staged during pair 3's attention) is added
            # on the DVE and Y goes out in fp16 (halves the DMA drain; the
            # host sums the two core partials in f32).  The last q2's
            # deferred softmax tail is emitted a few tiles in; only token
            # chunks 12..15 depend on it.
            for nt in range(2):
                wo_t = wo_ts[nt]
                for tc_ in range(16):
                    if nt == 0 and tc_ == 4 and pending_tail is not None:
                        pending_tail()
                        pending_tail = None
                    yps = ps.tile([128, 512], F32, tag="ps_proj", bufs=2)
                    for j in range(PAIRS):
                        nc.tensor.matmul(
                            yps[:, :],
                            aot[j][:, tc_ * 128:(tc_ + 1) * 128],
                            wo_t[:, j * 512:(j + 1) * 512],
                            start=(j == 0), stop=(j == PAIRS - 1),
                        )
                    y_sb = sb.tile([128, 512], FP16, tag="y", bufs=3)
                    with nc.allow_low_precision(reason="fp16 partial Y"):
                        nc.vector.tensor_copy(y_sb[:, :], yps[:, :])
                    eng = nc.sync if tc_ % 2 == 0 else nc.gpsimd
                    eng.dma_start(
                        out=Y[tc_ * 128:(tc_ + 1) * 128, nt * 512:(nt + 1) * 512],
                        in_=y_sb[:, :],
                    )

    _split_multi_waits(nc)
    return nc


_nc_cache = {}
_last_results = None


def _get_nc():
    if "nc" not in _nc_cache:
        _nc_cache["nc"] = build_bass()
    return _nc_cache["nc"]


def _prep_weights(hh, wq, bq, wk, bk, wv, bv, wo):
    """Per-head-half (hh in {0,1}) weight pack. Global pairs hh*4..hh*4+3."""
    wqT = np.ascontiguousarray(wq.T) * np.float32(1.0 / np.sqrt(DH))
    wkT = np.ascontiguousarray(wk.T)
    wvT = np.ascontiguousarray(wv.T)
    woT = np.ascontiguousarray(wo.T)
    jsl = slice(hh * PAIRS, (hh + 1) * PAIRS)
    csl = slice(hh * 512, (hh + 1) * 512)
    # WQP[j, p, (d m)] = wqT[d*128+p, (hh*4+j)*128+m]
    A = wqT.reshape(DINC, 128, 2 * PAIRS, 128)
    WQP = np.ascontiguousarray(
        A.transpose(2, 1, 0, 3)[jsl].reshape(PAIRS, 128, 1024)).astype(np.float16)
    A = wkT.reshape(DINC, 128, 2 * PAIRS, 128)
    WKP = np.ascontiguousarray(
        A.transpose(2, 1, 0, 3)[jsl].reshape(PAIRS, 128, 1024)).astype(np.float16)
    # WVP[p, (d n)] = wvT[d*128+p, hh*512+n]
    A = wvT[:, csl].reshape(DINC, 128, 512)
    WVP = np.ascontiguousarray(
        A.transpose(1, 0, 2).reshape(128, 4096)).astype(np.float16)
    # WOP[nt, p, (j n)] = woT[hh*512 + j*128+p, nt*512+n]
    A = woT[csl].reshape(PAIRS, 128, 2, 512)
    WOP = np.ascontiguousarray(
        A.transpose(2, 1, 0, 3).reshape(2, 128, 2048)).astype(np.float16)
    bqs = (bq * np.float32(1.0 / np.sqrt(DH))).reshape(2 * PAIRS, 128)[jsl]
    bkr = bk.reshape(2 * PAIRS, 128)[jsl]
    BQK = np.empty((128, 8), np.float32)
    for jx in range(PAIRS):
        BQK[:, 2 * jx] = bqs[jx]
        BQK[:, 2 * jx + 1] = bkr[jx]
    BVB = np.ascontiguousarray(np.tile(bv[csl].reshape(1, 512), (128, 1)))
    return {"WQP": WQP, "WKP": WKP, "WVP": WVP, "WOP": WOP,
            "BQK": BQK, "BVB": BVB}


def kernel(x_input, wq, bq, wk, bk, wv, bv, wo, bo):
    x_input = np.asarray(x_input, dtype=np.float32)
    wq, bq = np.asarray(wq, np.float32), np.asarray(bq, np.float32)
    wk, bk = np.asarray(wk, np.float32), np.asarray(bk, np.float32)
    wv, bv = np.asarray(wv, np.float32), np.asarray(bv, np.float32)
    wo, bo = np.asarray(wo, np.float32), np.asarray(bo, np.float32)

    packs = [_prep_weights(hh, wq, bq, wk, bk, wv, bv, wo) for hh in range(2)]
    ONES2D = np.ones((128, 128), np.float32)
    BMASK = np.zeros((33, 128), np.float32)
    BMASK[0, 0:64] = 1.0
    BMASK[32, 64:128] = 1.0
    xTs = [np.ascontiguousarray(x_input[b].T).astype(np.float16) for b in range(B)]

    nc = _get_nc()
    in_maps = []
    for c in range(N_CORES):
        b, hh = c // 2, c % 2
        m = dict(packs[hh])
        m["XT"] = xTs[b]
        m["ONES2D"] = ONES2D
        m["BMASK"] = BMASK
        in_maps.append(m)

    res = run_bass_kernel_spmd(nc, in_maps, list(range(N_CORES)))
    global _last_results
    _last_results = res

    out = np.empty((B, S, D), np.float32)
    for b in range(B):
        out[b] = res.results[2 * b]["Y"].astype(np.float32)
        out[b] += res.results[2 * b + 1]["Y"].astype(np.float32)
    out += bo.reshape(1, 1, D)
    return out


# revision 37
# speedup vs baseline: 1.0340x; 1.0118x over previous
"""Distributed MultiHeadAttention kernel for 8 TRN2 NeuronCores.

Problem: B=4, S=2048, D=1024, H=16, DH=64, fp32 reference, full
(non-causal) attention. ~137 GFLOP total.

Sharding (head-tensor-parallel x batch): core c owns batch b=c//2 and
head-half hh=c%2 (8 heads, full 2048-query x 2048-key attention). This
removes ALL duplicated work: each core projects q/k/v only for its 8
heads and contracts the output projection over its own 512 aot dims,
producing a partial Y[2048,1024] that the host sums pairwise
(Y(b) = Y(2b) + Y(2b+1) + bo). All 8 cores run ONE identical program;
only the DRAM inputs differ (batch xT and per-head-half weight slices).

Schedule notes (what made it fast, in order of impact):
- The attention inner loop is scalar(ACT)-bound: the per-key-chunk Exp
  [128,1024] takes ~1.1us while the 4 matmuls take ~0.86us. PV is
  software-pipelined ONE key chunk behind scores so the tensor queue
  never sits in the scores->exp->PV latency chain.
- The softmax reciprocal runs on the DVE (nc.vector.reciprocal), not as
  scalar Ln+Exp: the scalar engine is the attention bottleneck and the
  Ln table swap + 2 extra ACT ops per q2 cost more than DVE time.
- Pair j+1's Q/K projection matmuls are interleaved into pair j's
  attention stream (2 of 8 PSUM tiles after each q2 block) so the
  tensor engine uses the slack the scalar engine forces on it.
- aoT-normalize copies and Y staging copies run on GpSimd, which is
  otherwise idle after the x^T DMA; the DVE keeps the per-q2 tail
  (sum-row copies, reciprocal, normalize muls) short.
- fp8 was evaluated and rejected: absmax rel err tolerance (2e-2)
  needs per-element output noise sigma < 3e-3, and every fp8 placement
  (P/V, projections, or O-proj operands) measured 3e-2..9e-2 in a host
  simulation of the exact quantization chain.
- walrus in this environment rejects >1 semaphore wait per instruction;
  a post-pass hoists extra waits onto standalone same-engine
  InstEventSemaphore instructions.
"""
import numpy as np
import ml_dtypes
import concourse.bass as bass
import concourse.mybir as mybir
from concourse.tile import TileContext
from concourse.bass_utils import run_bass_kernel_spmd


def _ensure_trace_shim():
    """concourse's axon trace path imports antenv.axon_hooks, which this
    container's antenv lacks. Install a working ctypes-based NTFF hook (or a
    None hook) so BASS_TRACE=1 degrades gracefully instead of crashing."""
    try:
        import antenv.axon_hooks  # noqa: F401
        return
    except ImportError:
        pass
    import sys as _sys
    import types as _types
    hook = None
    try:
        if "/root/.axon_site" not in _sys.path:
            _sys.path.insert(0, "/root/.axon_site")
        from trn_agent_boot.trn_boot import _ntff_profile_via_ctypes
        hook = _ntff_profile_via_ctypes("/opt/axon/libaxon_pjrt.so")
    except Exception:
        hook = None
    mod = _types.ModuleType("antenv.axon_hooks")
    mod.get_axon_ntff_profile_hook = lambda: hook
    mod.set_axon_ntff_profile_hook = lambda h: None
    _sys.modules["antenv.axon_hooks"] = mod
    try:
        import concourse.bass_utils as _bu
        _bu.upload_artifacts = lambda tmpdir: f"local:{tmpdir}"
    except Exception:
        pass


_ensure_trace_shim()


F32 = mybir.dt.float32
F32R = mybir.dt.float32r
BF16 = mybir.dt.bfloat16
FP16 = mybir.dt.float16

B, S, D, H = 4, 2048, 1024, 16
DH = D // H
N_CORES = 8
NQ = S                     # 2048 queries per core (full sequence)
PAIRS = 4                  # head pairs per core (8 heads)
DINC = 8                   # 128-wide din chunks
KC = S // 128              # 16 key chunks
QT = NQ // 512             # 4 query tiles
SEG = 772                  # vaug cols per key chunk (4 pairs x 193)

_ws_counter = 0


def _split_multi_waits(nc):
    """walrus in this env rejects >1 sem wait per instruction; hoist extras
    onto same-engine standalone semaphore-wait instructions."""
    global _ws_counter
    f = nc.m.functions[0]
    for bb in f.blocks:
        insts = bb.instructions  # live list
        i = 0
        while i < len(insts):
            inst = insts[i]
            si = inst.sync_info
            waits = list(si.on_wait) if si is not None and si.on_wait else []
            if len(waits) > 1:
                eng = getattr(inst, "engine", None)
                assert eng is not None and eng in nc.engines, (
                    f"multi-wait on non-engine inst {inst.name} ({type(inst).__name__})"
                )
                for w in waits[:-1]:
                    _ws_counter += 1
                    ev = mybir.InstEventSemaphore(
                        name=f"I-wsplit-{_ws_counter}", ins=[], outs=[]
                    )
                    ev.engine = eng
                    ev.sync_info = mybir.SyncInfo(on_wait=[w], on_update=[])
                    nc.register_instruction(ev, overwrite=True)
                    insts.insert(i, ev)
                    i += 1
                inst.sync_info = mybir.SyncInfo(
                    on_wait=[waits[-1]], on_update=list(si.on_update or [])
                )
            i += 1


def build_bass():
    nc = bass.Bass()
    XT = nc.declare_dram_parameter("XT", [D, S], FP16, isOutput=False)
    # contiguous copy of x^T[:, 0:256] so the startup first-wave DMAs are
    # single-burst reads (XT rows are 4KB-strided -> 128 descriptors each)
    XTW = nc.declare_dram_parameter("XTW", [DINC, 128, 256], FP16, isOutput=False)
    WQP = nc.declare_dram_parameter("WQP", [PAIRS, 128, 1024], FP16, isOutput=False)
    WKP = nc.declare_dram_parameter("WKP", [PAIRS, 128, 1024], FP16, isOutput=False)
    WVP = nc.declare_dram_parameter("WVP", [128, 4096], FP16, isOutput=False)
    WOP = nc.declare_dram_parameter("WOP", [2, 128, 2048], FP16, isOutput=False)
    BQK = nc.declare_dram_parameter("BQK", [128, 8], F32, isOutput=False)
    BVB = nc.declare_dram_parameter("BVB", [128, 512], F32, isOutput=False)
    ONES2D = nc.declare_dram_parameter("ONES2D", [128, 128], F32, isOutput=False)
    # BMASK rows 0/32: partition-block masks so ONE K=33 matmul broadcasts
    # both reciprocal rows (rrec rows 0 and 32) to partitions 0:64 / 64:128.
    BMASK = nc.declare_dram_parameter("BMASK", [33, 128], F32, isOutput=False)
    Y = nc.declare_dram_parameter("Y", [NQ, D], FP16, isOutput=True)

    with TileContext(nc) as tc:
        with (
            tc.tile_pool(name="sb", bufs=1) as sb,
            tc.tile_pool(name="ps", bufs=1, space="PSUM") as ps,
        ):
            # ---- DMA priority: wv half 1 (first compute) on sync; x^T first
            # wave on gpsimd in parallel; everything else behind them.
            wv_t = sb.tile([128, 4096], FP16, tag="wv", name="wv_t")
            nc.sync.dma_start(out=wv_t[:, 0:2048], in_=WVP[:, 0:2048])

            xt = []
            for d in range(DINC):
                t = sb.tile([128, S], FP16, tag=f"xt{d}")
                nc.gpsimd.dma_start(out=t[:, 0:256], in_=XTW[d, :, :])
                xt.append(t)
            nc.sync.dma_start(out=wv_t[:, 2048:4096], in_=WVP[:, 2048:4096])

            ones2d = sb.tile([128, 128], F32R, tag="ones2d")
            bmask = sb.tile([33, 128], F32R, tag="bmask")
            bqk = sb.tile([128, 8], F32, tag="bqk")
            bvb = sb.tile([128, 512], F32, tag="bvb")
            nc.sync.dma_start(out=bvb[:, :], in_=BVB[:, :])
            nc.sync.dma_start(out=bqk[:, :], in_=BQK[:, :])
            nc.sync.dma_start(out=ones2d[:, :], in_=ONES2D[:, :].bitcast(F32R))
            nc.sync.dma_start(out=bmask[:, :], in_=BMASK[:, :].bitcast(F32R))

            for d in range(DINC):
                nc.gpsimd.dma_start(out=xt[d][:, 256:1024],
                                    in_=XT[d * 128:(d + 1) * 128, 256:1024])
            for d in range(DINC):
                nc.gpsimd.dma_start(out=xt[d][:, 1024:2048],
                                    in_=XT[d * 128:(d + 1) * 128, 1024:2048])

            # Q/K/O weights early on the sync queue (it is idle after startup)
            wq_ts, wk_ts = [], []
            for j in range(PAIRS):
                wq_t = sb.tile([128, 1024], FP16, tag=f"wq{j}")
                wk_t = sb.tile([128, 1024], FP16, tag=f"wk{j}")
                nc.sync.dma_start(out=wq_t[:, :], in_=WQP[j, :, :])
                nc.sync.dma_start(out=wk_t[:, :], in_=WKP[j, :, :])
                wq_ts.append(wq_t)
                wk_ts.append(wk_t)
            wo_ts = []
            for nt in range(2):
                wo_t = sb.tile([128, 2048], FP16, tag=f"wo{nt}")
                nc.sync.dma_start(out=wo_t[:, :], in_=WOP[nt, :, :])
                wo_ts.append(wo_t)

            aot = [sb.tile([128, NQ], FP16, tag=f"ao{j}", name=f"ao{j}")
                   for j in range(PAIRS)]
            qt_ts = [sb.tile([128, NQ], FP16, tag=f"qt{j}", name=f"qt{j}") for j in range(PAIRS)]
            kt_ts = [sb.tile([128, S], FP16, tag=f"kt{j}", name=f"kt{j}") for j in range(PAIRS)]

            # ---- V-projection into augmented per-pair layout: per key-chunk
            # segment of 772 cols, 4 pair-sub-segments of 193:
            #   [V_h0 64 | ones | ones | junk62 | V_h1 64]
            # augA = seg[0:65]  (M=65; psA row 64 = h0 softmax sums)
            # augB = seg[65:193] (col 0 ones -> psB row 0 = h1 sums;
            #                     cols 64:128 = V_h1 -> psB rows 64:127)
            vaug = sb.tile([128, KC * SEG], BF16, tag="vaug", name="vaug")
            vsegs = vaug[:, :].rearrange("p (s c) -> p s c", c=SEG)
            for jj in range(PAIRS):
                nc.vector.memset(vsegs[:, :, jj * 193 + 64:jj * 193 + 65], 1.0)
                nc.vector.memset(vsegs[:, :, jj * 193 + 65:jj * 193 + 66], 1.0)
            for kc in range(KC):
                vps = ps.tile([128, 512], F32, tag="ps_proj", bufs=2)
                for d in range(DINC):
                    nc.tensor.matmul(
                        vps[:, :],
                        xt[d][:, kc * 128:(kc + 1) * 128],
                        wv_t[:, d * 512:(d + 1) * 512],
                        start=(d == 0), stop=(d == DINC - 1),
                    )
                # merged strided adds: one op per head-position covers all 4
                # pair sub-segments (3D APs), 2 DVE ops per key chunk not 8
                s0 = kc * SEG
                vseg = vaug[:, s0:s0 + SEG].rearrange("p (j c) -> p j c", c=193)
                vpsq = vps[:, :].rearrange("p (j c) -> p j c", c=128)
                bvbq = bvb[:, :].rearrange("p (j c) -> p j c", c=128)
                with nc.allow_low_precision(reason="bf16 V"):
                    nc.vector.tensor_add(
                        vseg[:, :, 0:64], vpsq[:, :, 0:64], bvbq[:, :, 0:64])
                    nc.vector.tensor_add(
                        vseg[:, :, 129:193], vpsq[:, :, 64:128],
                        bvbq[:, :, 64:128])

            def qk_proj_tile(j, which, tt, eng=None):
                """One PSUM tile (1/8) of pair j's Q or K projection. The
                bias-add runs on GpSimd when interleaved into the attention
                stream: the DVE's 2-4.6us reciprocal blocks would otherwise
                delay the add, park the ps_proj buffer, and stall the whole
                (scheduler-interleaved) tensor queue."""
                w_t = wq_ts[j] if which == 0 else wk_ts[j]
                o_t = qt_ts[j] if which == 0 else kt_ts[j]
                pps = ps.tile([128, 512], F32, tag="ps_proj", bufs=2)
                for d in range(DINC):
                    nc.tensor.matmul(
                        pps[:, :],
                        w_t[:, d * 128:(d + 1) * 128],
                        xt[d][:, tt * 512:(tt + 1) * 512],
                        start=(d == 0), stop=(d == DINC - 1),
                    )
                with nc.allow_low_precision(reason="f32r rounding"):
                    (eng or nc.vector).tensor_scalar_add(
                        o_t[:, tt * 512:(tt + 1) * 512], pps[:, :],
                        bqk[:, 2 * j + which:2 * j + which + 1],
                    )

            # pair 0's full Q/K projection up front
            for which in range(2):
                for tt in range(4):
                    qk_proj_tile(0, which, tt)

            # ---- main loop over head pairs; pair j+1's projection is
            # interleaved (2 of its 8 PSUM tiles after each q2 block), and
            # each q2's softmax-tail broadcast+muls are DEFERRED into the
            # middle of the next q2's kc loop: by then the DVE reciprocal is
            # long finished, so the in-order tensor queue never waits on it.
            def make_tail(j, qsl, rrec, aocp):
                def emit():
                    # single-bank broadcast: ONE K=33 matmul with the BMASK
                    # stationary places 1/sums_h0 on partitions 0:64 and
                    # 1/sums_h1 on 64:128 (junk rrec rows hit zero mask
                    # rows).  psbc comes from the ps_proj pool: its rotation
                    # gives the muls ~10us before the bank is reused (the
                    # ps_s rotation would stall the next q2's scores).
                    psbc = ps.tile([128, 512], F32, tag="ps_proj", bufs=2,
                                   name="psbc")
                    nc.tensor.matmul(psbc[:, :], bmask[:, :], rrec[:, :],
                                     start=True, stop=True)
                    with nc.allow_low_precision(reason="fp16 out"):
                        nc.vector.tensor_mul(
                            aot[j][0:64, qsl], aocp[0:64, 0:512],
                            psbc[0:64, :],
                        )
                        nc.vector.tensor_mul(
                            aot[j][64:128, qsl], aocp[64:128, 512:1024],
                            psbc[64:128, :],
                        )
                return emit

            pending_tail = None
            for j in range(PAIRS):
                qt_t, kt_t = qt_ts[j], kt_ts[j]
                for q2 in range(QT):
                    psA = ps.tile([65, 512], F32, tag="ps_pv", bufs=2)
                    psB = ps.tile([128, 512], F32, tag="ps_pv", bufs=2)
                    qsl = slice(q2 * 512, (q2 + 1) * 512)
                    # software-pipelined: scores(kc) ... PV(kc-1); one matmul
                    # of the next pair's projection rides in each kc period
                    # (tile a over kc 1..8, tile b over kc 9..16).
                    pts = [None, None]
                    proj_plan = []
                    if j + 1 < PAIRS:
                        for half in range(2):
                            w_t = wq_ts[j + 1] if q2 < 2 else wk_ts[j + 1]
                            o_t = qt_ts[j + 1] if q2 < 2 else kt_ts[j + 1]
                            tt = 2 * (q2 % 2) + half
                            proj_plan.append(
                                (w_t, o_t, tt, 2 * (j + 1) + (0 if q2 < 2 else 1)))
                    proj_ps = [None]

                    def proj_step(step):
                        half, d = step // DINC, step % DINC
                        w_t, o_t, tt, bcol = proj_plan[half]
                        if d == 0:
                            proj_ps[0] = ps.tile([128, 512], F32,
                                                 tag="ps_proj", bufs=2,
                                                 name="pps")
                        nc.tensor.matmul(
                            proj_ps[0][:, :],
                            w_t[:, d * 128:(d + 1) * 128],
                            xt[d][:, tt * 512:(tt + 1) * 512],
                            start=(d == 0), stop=(d == DINC - 1),
                        )
                        if d == DINC - 1:
                            with nc.allow_low_precision(reason="f32r rounding"):
                                nc.vector.tensor_scalar_add(
                                    o_t[:, tt * 512:(tt + 1) * 512],
                                    proj_ps[0][:, :],
                                    bqk[:, bcol:bcol + 1],
                                )

                    for kc in range(KC + 1):
                        if kc == 6 and pending_tail is not None:
                            pending_tail()
                            pending_tail = None
                        if kc < KC:
                            pss = ps.tile([128, 1024], F32, tag="ps_s", bufs=2)
                            ksl = slice(kc * 128, (kc + 1) * 128)
                            nc.tensor.matmul(
                                pss[:, 0:512], kt_t[0:64, ksl], qt_t[0:64, qsl],
                                start=True, stop=True,
                            )
                            nc.tensor.matmul(
                                pss[:, 512:1024], kt_t[64:128, ksl],
                                qt_t[64:128, qsl],
                                start=True, stop=True,
                            )
                            pt = sb.tile([128, 1024], BF16, tag="pt", bufs=6)
                            nc.scalar.activation(
                                pt[:, :], pss[:, :],
                                mybir.ActivationFunctionType.Exp,
                            )
                            pts[kc % 2] = pt
                        if kc >= 1:
                            pv = kc - 1
                            pt_p = pts[pv % 2]
                            s0 = pv * SEG + j * 193
                            nc.tensor.matmul(
                                psA[:, :], vaug[:, s0:s0 + 65], pt_p[:, 0:512],
                                start=(pv == 0), stop=(pv == KC - 1),
                            )
                            nc.tensor.matmul(
                                psB[:, :], vaug[:, s0 + 65:s0 + 193],
                                pt_p[:, 512:1024],
                                start=(pv == 0), stop=(pv == KC - 1),
                            )
                            if proj_plan:
                                proj_step(kc - 1)

                    # DVE half of the softmax tail, emitted now: free psA/psB
                    # via the staging copies, then 1/sums on the DVE (the
                    # [33,512] reciprocal costs ~3.3us and runs during the
                    # next q2's early key chunks).  sums sit in psA row 64
                    # (h0, copied to row 0) / psB row 0 (h1, copied to row
                    # 32); srow rows 1..31 are memset to 1.0 once per buffer
                    # so the reciprocal reads no junk.
                    srow = sb.tile([33, 512], F32, tag="srow", bufs=2)
                    rrec = sb.tile([33, 512], F32R, tag="rrec", bufs=2)
                    aocp = sb.tile([128, 1024], F32, tag="aocp", bufs=2)
                    nc.vector.tensor_copy(aocp[0:64, 0:512], psA[0:64, :])
                    nc.vector.tensor_copy(aocp[64:128, 512:1024], psB[64:128, :])
                    if j == 0 and q2 < 2:
                        nc.vector.memset(srow[:, :], 1.0)
                    nc.vector.tensor_copy(srow[0:1, :], psA[64:65, :])
                    nc.vector.tensor_copy(srow[32:33, :], psB[0:1, :])
                    with nc.allow_low_precision(reason="softmax recip"):
                        nc.vector.reciprocal(out=rrec[:, :],
                                             in_=srow[:, :])
                    pending_tail = make_tail(j, qsl, rrec, aocp)

            # ---- output projection pass 2: pairs 2+3 accumulate in PSUM,
            # then y01 (pairs 0+1, staged during pair 3's attention) is added
            # on the DVE and Y goes out in fp16 (halves the DMA drain; the
            # host sums the two core partials in f32).  The last q2's
            # deferred softmax tail is emitted a few tiles in; only token
            # chunks 12..15 depend on it.
            for nt in range(2):
                wo_t = wo_ts[nt]
                for tc_ in range(16):
                    if nt == 0 and tc_ == 4 and pending_tail is not None:
                        pending_tail()
                        pending_tail = None
                    yps = ps.tile([128, 512], F32, tag="ps_proj", bufs=2)
                    for j in range(PAIRS):
                        nc.tensor.matmul(
                            yps[:, :],
                            aot[j][:, tc_ * 128:(tc_ + 1) * 128],
                            wo_t[:, j * 512:(j + 1) * 512],
                            start=(j == 0), stop=(j == PAIRS - 1),
                        )
                    y_sb = sb.tile([128, 512], FP16, tag="y", bufs=3)
                    with nc.allow_low_precision(reason="fp16 partial Y"):
                        nc.vector.tensor_copy(y_sb[:, :], yps[:, :])
                    eng = nc.sync if tc_ % 2 == 0 else nc.gpsimd
                    eng.dma_start(
                        out=Y[tc_ * 128:(tc_ + 1) * 128, nt * 512:(nt + 1) * 512],
                        in_=y_sb[:, :],
                    )

    _split_multi_waits(nc)
    return nc


_nc_cache = {}
_last_results = None


def _get_nc():
    if "nc" not in _nc_cache:
        _nc_cache["nc"] = build_bass()
    return _nc_cache["nc"]


def _prep_weights(hh, wq, bq, wk, bk, wv, bv, wo):
    """Per-head-half (hh in {0,1}) weight pack. Global pairs hh*4..hh*4+3."""
    wqT = np.ascontiguousarray(wq.T) * np.float32(1.0 / np.sqrt(DH))
    wkT = np.ascontiguousarray(wk.T)
    wvT = np.ascontiguousarray(wv.T)
    woT = np.ascontiguousarray(wo.T)
    jsl = slice(hh * PAIRS, (hh + 1) * PAIRS)
    csl = slice(hh * 512, (hh + 1) * 512)
    # WQP[j, p, (d m)] = wqT[d*128+p, (hh*4+j)*128+m]
    A = wqT.reshape(DINC, 128, 2 * PAIRS, 128)
    WQP = np.ascontiguousarray(
        A.transpose(2, 1, 0, 3)[jsl].reshape(PAIRS, 128, 1024)).astype(np.float16)
    A = wkT.reshape(DINC, 128, 2 * PAIRS, 128)
    WKP = np.ascontiguousarray(
        A.transpose(2, 1, 0, 3)[jsl].reshape(PAIRS, 128, 1024)).astype(np.float16)
    # WVP[p, (d n)] = wvT[d*128+p, hh*512+n]
    A = wvT[:, csl].reshape(DINC, 128, 512)
    WVP = np.ascontiguousarray(
        A.transpose(1, 0, 2).reshape(128, 4096)).astype(np.float16)
    # WOP[nt, p, (j n)] = woT[hh*512 + j*128+p, nt*512+n]
    A = woT[csl].reshape(PAIRS, 128, 2, 512)
    WOP = np.ascontiguousarray(
        A.transpose(2, 1, 0, 3).reshape(2, 128, 2048)).astype(np.float16)
    bqs = (bq * np.float32(1.0 / np.sqrt(DH))).reshape(2 * PAIRS, 128)[jsl]
    bkr = bk.reshape(2 * PAIRS, 128)[jsl]
    BQK = np.empty((128, 8), np.float32)
    for jx in range(PAIRS):
        BQK[:, 2 * jx] = bqs[jx]
        BQK[:, 2 * jx + 1] = bkr[jx]
    BVB = np.ascontiguousarray(np.tile(bv[csl].reshape(1, 512), (128, 1)))
    return {"WQP": WQP, "WKP": WKP, "WVP": WVP, "WOP": WOP,
            "BQK": BQK, "BVB": BVB}


def kernel(x_input, wq, bq, wk, bk, wv, bv, wo, bo):
    x_input = np.asarray(x_input, dtype=np.float32)
    wq, bq = np.asarray(wq, np.float32), np.asarray(bq, np.float32)
    wk, bk = np.asarray(wk, np.float32), np.asarray(bk, np.float32)
    wv, bv = np.asarray(wv, np.float32), np.asarray(bv, np.float32)
    wo, bo = np.asarray(wo, np.float32), np.asarray(bo, np.float32)

    packs = [_prep_weights(hh, wq, bq, wk, bk, wv, bv, wo) for hh in range(2)]
    ONES2D = np.ones((128, 128), np.float32)
    BMASK = np.zeros((33, 128), np.float32)
    BMASK[0, 0:64] = 1.0
    BMASK[32, 64:128] = 1.0
    xTs = [np.ascontiguousarray(x_input[b].T).astype(np.float16) for b in range(B)]

    XTWs = [np.ascontiguousarray(
        xTs[b][:, 0:256].reshape(DINC, 128, 256)) for b in range(B)]

    nc = _get_nc()
    in_maps = []
    for c in range(N_CORES):
        b, hh = c // 2, c % 2
        m = dict(packs[hh])
        m["XT"] = xTs[b]
        m["XTW"] = XTWs[b]
        m["ONES2D"] = ONES2D
        m["BMASK"] = BMASK
        in_maps.append(m)

    res = run_bass_kernel_spmd(nc, in_maps, list(range(N_CORES)))
    global _last_results
    _last_results = res

    out = np.empty((B, S, D), np.float32)
    for b in range(B):
        out[b] = res.results[2 * b]["Y"].astype(np.float32)
        out[b] += res.results[2 * b + 1]["Y"].astype(np.float32)
    out += bo.reshape(1, 1, D)
    return out
